# revision 2
# baseline (speedup 1.0000x reference)
"""CFNet interaction block on 8 TRN2 NeuronCores (Bass/Tile).

Strategy (self-contained; shapes hardcoded for this problem):
  - seg_j == arange(E) so the first segment_sum is the identity: w_ij = w_ijk.
  - Shard interactions (E=800000) across 8 cores at seg_i segment boundaries
    so each core owns a contiguous atom band; no collectives.
  - Per core, the band is processed as NW windows of 128 atoms. Windows are
    SORTED by edge-count (descending) into capacity slots shared across all
    cores (capacity = max over cores per slot) -> ~3% padding instead of ~13%.
    Host un-permutes the outputs.
  - ssp(x) = ln(0.5*exp(x) + 0.5) via ACT Exp then Ln with the free affine
    pre-transform (scale=0.5, bias=0.5); exact incl. the -log2. (This
    toolchain's gen3 activation tables have no Softplus.) The Exp writes
    bf16 to SBUF and the Ln runs SBUF->SBUF, both 1024 wide to amortize
    the ~222-cycle ACT instruction overhead.
  - Segment-sum via onehot matmuls into per-window PSUM accumulators; the
    onehot (is_equal vs iota) runs on the Pool engine (otherwise idle).
  - Software-pipelined emission: per step s the engines work on different
    groups (DMA s | mm1/sp1/hsub s-1 | mm2/sp2/wf s-2 | scatter s-3) so no
    engine queue head-blocks on the cross-engine dependency chain.
"""
import os
import sys
import numpy as np

sys.path.insert(0, "/opt/trn_rl_repo")

import ml_dtypes

import concourse.bass as bass
import concourse.mybir as mybir
import concourse.tile as tile
from concourse import bacc
import concourse.bass_utils as bass_utils
from concourse.bass_utils import run_bass_kernel_spmd

# ---- disable walrus birsim (compile-time only; no effect on generated code) ----
_orig_run_command = bass_utils.run_command


def _patched_run_command(argv, **kwargs):
    argv = [a.replace("--enable-birsim=true", "--enable-birsim=false")
            if isinstance(a, str) else a for a in argv]
    return _orig_run_command(argv, **kwargs)


bass_utils.run_command = _patched_run_command

# ---- activation-table selection fix ----------------------------------------
# The act-table insertion pass picks the FIRST table set containing each
# activation's function; with Exp and Ln alternating it ping-pongs between
# 'exp_and_others' and 'natural_log' (one ~1.3us ACT table load per swap,
# hundreds per iteration). Strip Exp/Ln from every set except
# 'natural_log_exp_and_others' (which genuinely contains both splines) so the
# pass settles on that one set; dict order (= act_func_set_id) is preserved.
import concourse.bacc as _bacc_mod
from concourse.hw_specs import get_activation_tables as _orig_gat


def _gat_prefer_dual(arch):
    t = _orig_gat(arch)
    AF_ = mybir.ActivationFunctionType
    out = {}
    for name, fns in t.items():
        if name != "natural_log_exp_and_others":
            fns = set(fns) - {AF_.Exp, AF_.Ln}
        out[name] = set(fns)
    return out


_bacc_mod.get_activation_tables = _gat_prefer_dual

P = 128
NCORES = 8
N_ATOMS = 50000
NFM = 128
LOG2 = float(np.log(2.0))
G = 8          # tiles per group
GW = G * P     # ints per group (1024)

F32 = mybir.dt.float32
BF16 = mybir.dt.bfloat16
AF = mybir.ActivationFunctionType
ALU = mybir.AluOpType

_cache = {}


def _build_nc(NW, NTILE, FLUSH, repeat=1, onehot_engine="vector"):
    """SPMD program. NW window-slots of 128 atoms per core; NTILE tiles of
    128 interactions (divisible by G); FLUSH = tuple of cumulative tile
    boundaries per slot (len NW+1, F[0]=0, F[NW]=NTILE)."""
    key = (NW, NTILE, FLUSH, repeat, onehot_engine)
    if key in _cache:
        return _cache[key]

    assert NTILE % G == 0
    NG = NTILE // G
    BAND = NW * P

    nc = bacc.Bacc("TRN2", target_bir_lowering=False, debug=False,
                   num_devices=NCORES)

    dijk_d = nc.dram_tensor("dijk", [NG, P, GW], BF16, kind="ExternalInput")
    xj_d = nc.dram_tensor("xj", [NG, P, GW], BF16, kind="ExternalInput")
    segl_d = nc.dram_tensor("segl", [P, NTILE], BF16, kind="ExternalInput")
    xb_d = nc.dram_tensor("xb", [P, NW, P], BF16, kind="ExternalInput")
    w1_d = nc.dram_tensor("w1", [P, P], BF16, kind="ExternalInput")
    w2_d = nc.dram_tensor("w2", [P, P], BF16, kind="ExternalInput")
    wf2o_d = nc.dram_tensor("wf2o", [P, P], BF16, kind="ExternalInput")
    wd_d = nc.dram_tensor("wd", [P, P], BF16, kind="ExternalInput")
    b1_d = nc.dram_tensor("b1", [P, 1], F32, kind="ExternalInput")
    bf2o_d = nc.dram_tensor("bf2o", [P, 1], F32, kind="ExternalInput")
    bdp_d = nc.dram_tensor("bdp", [P, P], F32, kind="ExternalInput")
    iota_d = nc.dram_tensor("iota", [P, P], BF16, kind="ExternalInput")

    y_d = nc.dram_tensor("y", [P, NW, P], BF16, kind="ExternalOutput")
    v_d = nc.dram_tensor("v", [P, NW, P], BF16, kind="ExternalOutput")

    # slot index per tile (compile-time)
    slot_of = np.searchsorted(np.asarray(FLUSH), np.arange(NTILE),
                              side="right") - 1

    with tile.TileContext(nc) as tc:
        with tc.tile_pool(name="const", bufs=1) as cpool, \
             tc.tile_pool(name="io", bufs=1) as iop, \
             tc.tile_pool(name="mid", bufs=1) as midp, \
             tc.tile_pool(name="ps", bufs=1, space="PSUM") as psp:

            # ---- constants (loaded once, outside the repeat loop) ----
            w1_s = cpool.tile([P, P], BF16)
            nc.sync.dma_start(out=w1_s[:], in_=w1_d[:, :])
            w2_s = cpool.tile([P, P], BF16)
            nc.sync.dma_start(out=w2_s[:], in_=w2_d[:, :])
            wf2o_s = cpool.tile([P, P], BF16)
            nc.sync.dma_start(out=wf2o_s[:], in_=wf2o_d[:, :])
            wd_s = cpool.tile([P, P], BF16)
            nc.sync.dma_start(out=wd_s[:], in_=wd_d[:, :])
            b1_s = cpool.tile([P, 1], F32)
            nc.sync.dma_start(out=b1_s[:], in_=b1_d[:, :])
            bf2o_s = cpool.tile([P, 1], F32)
            nc.sync.dma_start(out=bf2o_s[:], in_=bf2o_d[:, :])
            bdp_s = cpool.tile([P, P], F32)
            nc.sync.dma_start(out=bdp_s[:], in_=bdp_d[:, :])
            iota_s = cpool.tile([P, P], BF16)
            nc.sync.dma_start(out=iota_s[:], in_=iota_d[:, :])
            segl_s = cpool.tile([P, NTILE], BF16)
            nc.sync.dma_start(out=segl_s[:], in_=segl_d[:, :])
            xb_s = cpool.tile([P, NW, P], BF16)
            nc.sync.dma_start(out=xb_s[:], in_=xb_d[:, :, :])
            half_s = cpool.tile([P, 1], F32)
            nc.vector.memset(half_s[:], 0.5)

            convT = cpool.tile([P, BAND], BF16)

            oh_eng = nc.gpsimd if onehot_engine == "gpsimd" else nc.vector

            def body():
                dks, xjs, hs, wsbs, wfs, ohs = {}, {}, {}, {}, {}, {}
                cvs = {}

                def dma_in(s):
                    dk = iop.tile([P, GW], BF16, tag="dk", bufs=3)
                    nc.sync.dma_start(out=dk[:], in_=dijk_d[s])
                    xjt = iop.tile([P, GW], BF16, tag="xj", bufs=5)
                    nc.sync.dma_start(out=xjt[:], in_=xj_d[s])
                    dks[s], xjs[s] = dk, xjt

                def onehot(s):
                    oh = midp.tile([P, G, P], BF16, tag="oh", bufs=5)
                    tl = s * G
                    oh_eng.tensor_tensor(
                        out=oh[:],
                        in0=segl_s[:, tl:tl + G].unsqueeze(2)
                            .to_broadcast([P, G, P]),
                        in1=iota_s[:].unsqueeze(1).to_broadcast([P, G, P]),
                        op=ALU.is_equal)
                    ohs[s] = oh

                def stage_mm_act(s1, s2):
                    """mm1 for group s1 and mm2 for group s2 share one PSUM
                    tile; one Exp + one Ln cover both (2048 wide)."""
                    a12 = psp.tile([P, 2 * GW], F32, tag="a12", bufs=1)
                    lo, hi = (0 if s1 is not None else GW,
                              2 * GW if s2 is not None else GW)
                    if s1 is not None:
                        dk = dks[s1]
                        for hf in range(2):
                            c0 = hf * 512
                            nc.tensor.matmul(out=a12[:, c0:c0 + 512],
                                             lhsT=w1_s[:],
                                             rhs=dk[:, c0:c0 + 512],
                                             start=True, stop=True)
                    if s2 is not None:
                        h = hs[s2]
                        for c in range(G):
                            nc.tensor.matmul(
                                out=a12[:, GW + c * P:GW + (c + 1) * P],
                                lhsT=h[0][:, h[1] + c * P:h[1] + (c + 1) * P],
                                rhs=w2_s[:], start=True, stop=True)
                        del hs[s2]
                    e12 = midp.tile([P, 2 * GW], BF16, tag="e12", bufs=2)
                    nc.scalar.activation(out=e12[:, lo:hi], in_=a12[:, lo:hi],
                                         func=AF.Exp)
                    hw12 = midp.tile([P, 2 * GW], BF16, tag="hw12", bufs=4)
                    nc.scalar.activation(out=hw12[:, lo:hi],
                                         in_=e12[:, lo:hi],
                                         func=AF.Ln, scale=0.5,
                                         bias=half_s[:, :1])
                    if s1 is not None:
                        hs[s1] = (hw12, 0)       # h(s1) = hw12[:, :GW]
                    if s2 is not None:
                        wsbs[s2] = (hw12, GW)    # w(s2) = hw12[:, GW:]

                def wfmul(s):
                    wf = midp.tile([P, GW], BF16, tag="wf", bufs=3)
                    w = wsbs[s]
                    nc.vector.tensor_tensor(
                        out=wf[:], in0=w[0][:, w[1]:w[1] + GW],
                        in1=xjs[s][:], op=ALU.mult)
                    wfs[s] = wf
                    del wsbs[s], xjs[s]

                def scatter(s):
                    wf, oh = wfs[s], ohs[s]
                    for c in range(G):
                        t = s * G + c
                        sl = int(slot_of[t])
                        if t == FLUSH[sl]:
                            cvs[sl] = psp.tile([P, P], F32, tag="cv",
                                               bufs=2, name=f"cv{sl % 2}")
                        nc.tensor.matmul(out=cvs[sl][:],
                                         lhsT=wf[:, c * P:(c + 1) * P],
                                         rhs=oh[:, c, :],
                                         start=(t == FLUSH[sl]),
                                         stop=(t == FLUSH[sl + 1] - 1))
                        if t == FLUSH[sl + 1] - 1:
                            nc.vector.tensor_copy(
                                out=convT[:, sl * P:(sl + 1) * P],
                                in_=cvs[sl][:])
                            del cvs[sl]
                    del wfs[s], ohs[s]

                for s in range(NG + 4):
                    if s < NG:
                        dma_in(s)
                    if 0 <= s - 1 < NG:
                        onehot(s - 1)
                    # mm2 consumes h from two steps back so Exp12(s) never
                    # waits on the previous step's Ln12 via mm2.
                    s1 = s - 1 if 0 <= s - 1 < NG else None
                    s2 = s - 3 if 0 <= s - 3 < NG else None
                    if s1 is not None or s2 is not None:
                        stage_mm_act(s1, s2)
                    if s2 is not None:
                        wfmul(s2)
                    if 0 <= s - 4 < NG:
                        scatter(s - 4)

                # ---- atom stage ----
                for m0 in range(0, NW, G):
                    mw = min(G, NW - m0)
                    aw = mw * P
                    u = psp.tile([P, GW], F32, tag="a12", bufs=1)
                    for c0 in range(0, aw, 512):
                        cw = min(512, aw - c0)
                        nc.tensor.matmul(
                            out=u[:, c0:c0 + cw], lhsT=wf2o_s[:],
                            rhs=convT[:, m0 * P + c0:m0 * P + c0 + cw],
                            start=True, stop=True)
                    ec = midp.tile([P, GW], BF16, tag="ec", bufs=2)
                    nc.scalar.activation(out=ec[:, :aw], in_=u[:, :aw],
                                         func=AF.Exp, bias=bf2o_s[:, :1])
                    cT = midp.tile([P, GW], BF16, tag="cT", bufs=2)
                    nc.scalar.activation(out=cT[:, :aw], in_=ec[:, :aw],
                                         func=AF.Ln, scale=0.5,
                                         bias=half_s[:, :1])
                    vps = psp.tile([P, G, P], F32, tag="a2", bufs=1)
                    for c in range(mw):
                        nc.tensor.matmul(out=vps[:, c, :],
                                         lhsT=cT[:, c * P:(c + 1) * P],
                                         rhs=wd_s[:], start=True, stop=True)
                    vsb = midp.tile([P, G, P], BF16, tag="vsb", bufs=2)
                    nc.vector.tensor_tensor(
                        out=vsb[:, :mw, :], in0=vps[:, :mw, :],
                        in1=bdp_s[:].unsqueeze(1).to_broadcast([P, mw, P]),
                        op=ALU.add)
                    ysb = midp.tile([P, G, P], BF16, tag="ysb", bufs=2)
                    nc.vector.tensor_tensor(
                        out=ysb[:, :mw, :], in0=vps[:, :mw, :],
                        in1=xb_s[:, m0:m0 + mw, :],
                        op=ALU.add)
                    nc.sync.dma_start(
                        out=v_d[:, m0:m0 + mw, :],
                        in_=vsb[:, :mw, :])
                    nc.sync.dma_start(
                        out=y_d[:, m0:m0 + mw, :],
                        in_=ysb[:, :mw, :])

            if repeat == 1:
                body()
            else:
                with tc.For_i(0, repeat, 1):
                    body()

    nc.compile()
    _cache[key] = nc
    return nc


def _preprocess(x, f, dijk, idx_j, seg_i):
    """Host-side sharding. Returns (in_maps, meta)."""
    E = dijk.shape[0]
    seg_i = np.asarray(seg_i, dtype=np.int64)
    idx_j = np.asarray(idx_j, dtype=np.int64)

    # atom split points at segment boundaries (equal-edge quantiles)
    a_splits = [0]
    for k in range(1, NCORES):
        a_splits.append(int(seg_i[min(k * E // NCORES, E - 1)]))
    a_splits.append(N_ATOMS)
    for k in range(1, len(a_splits)):
        a_splits[k] = max(a_splits[k], a_splits[k - 1])
    e_bounds = [int(np.searchsorted(seg_i, a)) for a in a_splits]
    bands = [(a_splits[k], a_splits[k + 1] - a_splits[k])
             for k in range(NCORES)]
    NW = max(1, max((bl + P - 1) // P for _, bl in bands))

    # per core, per window: edge ranges & tile counts
    w_edges = np.zeros((NCORES, NW, 2), dtype=np.int64)
    w_tiles = np.zeros((NCORES, NW), dtype=np.int64)
    for k in range(NCORES):
        a0, _ = bands[k]
        e0, e1 = e_bounds[k], e_bounds[k + 1]
        seg_k = seg_i[e0:e1]
        for w in range(NW):
            lo, hi = a0 + w * P, a0 + (w + 1) * P
            s = int(np.searchsorted(seg_k, lo))
            e = int(np.searchsorted(seg_k, hi))
            w_edges[k, w] = (e0 + s, e0 + e)
            w_tiles[k, w] = (e - s + P - 1) // P

    # sort windows per core by tile count desc; capacity = per-slot max
    order = np.argsort(-w_tiles, axis=1, kind="stable")  # [NCORES, NW]
    sorted_tiles = np.take_along_axis(w_tiles, order, axis=1)
    C = np.maximum(1, sorted_tiles.max(axis=0))
    NTILE = int(C.sum())
    C[-1] += (-NTILE) % G
    NTILE = int(C.sum())
    FLUSH = tuple(int(v) for v in np.concatenate([[0], np.cumsum(C)]))

    E_pad = NTILE * P
    NG = NTILE // G
    in_maps = []
    metas = []
    for k in range(NCORES):
        a0, bl = bands[k]
        oc = np.zeros(E_pad, dtype=np.int64)
        valid = np.zeros(E_pad, dtype=bool)
        wbase = np.zeros(E_pad, dtype=np.int64)  # window base atom per slot
        for sl in range(NW):
            w = int(order[k, sl])
            es, ee = int(w_edges[k, w, 0]), int(w_edges[k, w, 1])
            n = ee - es
            o = FLUSH[sl] * P
            oc[o:o + n] = np.arange(es, ee)
            valid[o:o + n] = True
            wbase[o:FLUSH[sl + 1] * P] = a0 + w * P

        dmat = dijk[oc]
        dmat[~valid] = 0.0
        dijk_blk = np.ascontiguousarray(
            dmat.reshape(NG, GW, P).transpose(0, 2, 1)
        ).astype(ml_dtypes.bfloat16)

        xmat = f[idx_j[oc]]
        xmat[~valid] = 0.0
        xj_blk = np.ascontiguousarray(
            xmat.reshape(NG, G, P, P).transpose(0, 2, 1, 3).reshape(NG, P, GW)
        ).astype(ml_dtypes.bfloat16)

        segl_flat = np.where(valid, seg_i[oc] - wbase, -1).astype(np.float32)
        segl = np.ascontiguousarray(
            segl_flat.reshape(NTILE, P).T).astype(ml_dtypes.bfloat16)

        in_maps.append({"dijk": dijk_blk, "xj": xj_blk, "segl": segl})
        metas.append((a0, bl, order[k]))
    return in_maps, metas, NW, NTILE, FLUSH


def prepare(x, dijk, idx_j, seg_i, seg_j, seg_i_sum,
            W_f1, b_f1, W_f2, b_f2,
            W_in2fac, W_fac2out, b_fac2out,
            W_dense, b_dense):
    x = np.asarray(x, dtype=np.float32)
    dijk = np.asarray(dijk, dtype=np.float32)

    assert not np.any(np.asarray(b_f2)), \
        "b_f2 != 0 not supported by this build"
    assert not np.any(np.asarray(b_f1)), \
        "b_f1 != 0 not supported by this build (paired Exp has no bias slot)"

    f = x @ np.asarray(W_in2fac, dtype=np.float32)
    in_maps, metas, NW, NTILE, FLUSH = _preprocess(x, f, dijk, idx_j, seg_i)

    W_dense = np.asarray(W_dense, dtype=np.float32)
    bdp = np.asarray(b_dense, dtype=np.float32)

    consts = {
        "w1": np.asarray(W_f1, np.float32).astype(ml_dtypes.bfloat16),
        "w2": np.asarray(W_f2, np.float32).astype(ml_dtypes.bfloat16),
        "wf2o": np.asarray(W_fac2out, np.float32).astype(ml_dtypes.bfloat16),
        "wd": W_dense.astype(ml_dtypes.bfloat16),
        "b1": np.asarray(b_f1, np.float32).reshape(P, 1),
        "bf2o": np.asarray(b_fac2out, np.float32).reshape(P, 1),
        "bdp": np.broadcast_to(bdp[None, :], (P, P)).copy(),
        "iota": np.broadcast_to(
            np.arange(P, dtype=np.float32)[None, :],
            (P, P)).astype(ml_dtypes.bfloat16).copy(),
    }
    for k, m in enumerate(in_maps):
        a0, bl, order_k = metas[k]
        # xb[p, sl, j] = x[a0 + 128*order[sl] + p, j] + bdp[j]
        xb = np.zeros((P, NW, P), dtype=np.float32)
        for sl in range(NW):
            w = int(order_k[sl])
            r0 = a0 + w * P
            n = min(P, a0 + bl - r0)
            if n > 0:
                xb[:n, sl, :] = x[r0:r0 + n] + bdp[None, :]
        m["xb"] = xb.astype(ml_dtypes.bfloat16)
        m.update(consts)
    return (in_maps, metas, NW, NTILE, FLUSH)


def run_prepared(prepared, _repeat=1):
    in_maps, metas, NW, NTILE, FLUSH = prepared
    nc = _build_nc(NW, NTILE, FLUSH, repeat=_repeat)
    res = run_bass_kernel_spmd(nc, in_maps, core_ids=list(range(NCORES)))

    y = np.empty((N_ATOMS, P), dtype=np.float32)
    v = np.empty((N_ATOMS, P), dtype=np.float32)
    for k, (a0, bl, order_k) in enumerate(metas):
        yk = np.asarray(res.results[k]["y"], dtype=np.float32)
        vk = np.asarray(res.results[k]["v"], dtype=np.float32)
        for sl in range(NW):
            w = int(order_k[sl])
            r0 = a0 + w * P
            n = min(P, a0 + bl - r0)
            if n > 0:
                y[r0:r0 + n] = yk[:n, sl, :]
                v[r0:r0 + n] = vk[:n, sl, :]
    return (y, v)


def kernel(**inputs):
    return run_prepared(prepare(**inputs))


# revision 3
# speedup vs baseline: 1.3863x; 1.3863x over previous
"""CFNet interaction block on 8 TRN2 NeuronCores (Bass/Tile).

Strategy (self-contained; shapes hardcoded for this problem):
  - seg_j == arange(E) so the first segment_sum is the identity: w_ij = w_ijk.
  - The 391 global 128-atom windows are dealt round-robin (descending edge
    count) across the 8 cores, so per-slot tile capacities (cross-core max)
    hug the load distribution (~4% padding) and cores stay balanced; each
    core owns its windows' atoms outright -> no collectives. Host
    un-permutes the outputs.
  - ssp(x) = ln(0.5*exp(x) + 0.5) via ACT Exp then Ln with the free affine
    pre-transform (scale=0.5, bias=0.5) — exact incl. the -log2 (this
    toolchain's activation tables have no Softplus; its act2 slot resolves
    to a parametric linear, verified on HW).
  - Activation-table fix: without it the act-table-load pass ping-pongs
    between the Exp-only and Ln-only sets (~270us/iteration of table loads).
  - Stage-1 mm1 output and stage-2 mm2 output share one 2048-wide PSUM
    tile; a single Exp covers both, and the matching Ln runs one step later
    so consecutive ACT ops never chain RAW. ACT is the bottleneck engine
    (~90% busy).
  - Segment-sum via onehot (is_equal vs iota, DVE) + per-tile matmuls
    accumulating into per-window-slot PSUM; slot flushes at compile-time
    tile indices (uniform across cores). The atom stage (fac2out/dense/
    residual) is interleaved into the window loop as slots flush.
  - Software-pipelined emission with a 5-deep stage skew so no engine
    queue head-blocks on the cross-engine dependency chain.
"""
import os
import sys
import numpy as np

sys.path.insert(0, "/opt/trn_rl_repo")

import ml_dtypes

import concourse.bass as bass
import concourse.mybir as mybir
import concourse.tile as tile
from concourse import bacc
import concourse.bass_utils as bass_utils
from concourse.bass_utils import run_bass_kernel_spmd

# ---- disable walrus birsim (compile-time only; no effect on generated code) ----
_orig_run_command = bass_utils.run_command


def _patched_run_command(argv, **kwargs):
    argv = [a.replace("--enable-birsim=true", "--enable-birsim=false")
            if isinstance(a, str) else a for a in argv]
    return _orig_run_command(argv, **kwargs)


bass_utils.run_command = _patched_run_command

# ---- activation-table selection fix ----------------------------------------
# The act-table insertion pass picks the FIRST table set containing each
# activation's function; with Exp and Ln alternating it ping-pongs between
# 'exp_and_others' and 'natural_log' (one ~1.3us ACT table load per swap,
# hundreds per iteration). Strip Exp/Ln from every set except
# 'natural_log_exp_and_others' (which genuinely contains both splines) so the
# pass settles on that one set; dict order (= act_func_set_id) is preserved.
import concourse.bacc as _bacc_mod
from concourse.hw_specs import get_activation_tables as _orig_gat


def _gat_prefer_dual(arch):
    t = _orig_gat(arch)
    AF_ = mybir.ActivationFunctionType
    out = {}
    for name, fns in t.items():
        if name != "natural_log_exp_and_others":
            fns = set(fns) - {AF_.Exp, AF_.Ln}
        out[name] = set(fns)
    return out


_bacc_mod.get_activation_tables = _gat_prefer_dual

P = 128
NCORES = 8
N_ATOMS = 50000
NFM = 128
LOG2 = float(np.log(2.0))
G = 8          # tiles per group
GW = G * P     # ints per group (1024)

F32 = mybir.dt.float32
BF16 = mybir.dt.bfloat16
AF = mybir.ActivationFunctionType
ALU = mybir.AluOpType

_cache = {}


def _build_nc(NW, NTILE, FLUSH, repeat=1, onehot_engine="vector"):
    """SPMD program. NW window-slots of 128 atoms per core; NTILE tiles of
    128 interactions (divisible by G); FLUSH = tuple of cumulative tile
    boundaries per slot (len NW+1, F[0]=0, F[NW]=NTILE)."""
    key = (NW, NTILE, FLUSH, repeat, onehot_engine)
    if key in _cache:
        return _cache[key]

    assert NTILE % G == 0
    NG = NTILE // G
    BAND = NW * P

    nc = bacc.Bacc("TRN2", target_bir_lowering=False, debug=False,
                   num_devices=NCORES)

    dijk_d = nc.dram_tensor("dijk", [NG, P, GW], BF16, kind="ExternalInput")
    xj_d = nc.dram_tensor("xj", [NG, P, GW], BF16, kind="ExternalInput")
    segl_d = nc.dram_tensor("segl", [P, NTILE], BF16, kind="ExternalInput")
    xb_d = nc.dram_tensor("xb", [P, NW, P], BF16, kind="ExternalInput")
    w1_d = nc.dram_tensor("w1", [P, P], BF16, kind="ExternalInput")
    w2_d = nc.dram_tensor("w2", [P, P], BF16, kind="ExternalInput")
    wf2o_d = nc.dram_tensor("wf2o", [P, P], BF16, kind="ExternalInput")
    wd_d = nc.dram_tensor("wd", [P, P], BF16, kind="ExternalInput")
    b1_d = nc.dram_tensor("b1", [P, 1], F32, kind="ExternalInput")
    bf2o_d = nc.dram_tensor("bf2o", [P, 1], F32, kind="ExternalInput")
    bdp_d = nc.dram_tensor("bdp", [P, P], F32, kind="ExternalInput")
    iota_d = nc.dram_tensor("iota", [P, P], BF16, kind="ExternalInput")

    y_d = nc.dram_tensor("y", [P, NW, P], BF16, kind="ExternalOutput")
    v_d = nc.dram_tensor("v", [P, NW, P], BF16, kind="ExternalOutput")

    # slot index per tile (compile-time)
    slot_of = np.searchsorted(np.asarray(FLUSH), np.arange(NTILE),
                              side="right") - 1

    with tile.TileContext(nc) as tc:
        with tc.tile_pool(name="const", bufs=1) as cpool, \
             tc.tile_pool(name="io", bufs=1) as iop, \
             tc.tile_pool(name="mid", bufs=1) as midp, \
             tc.tile_pool(name="ps", bufs=1, space="PSUM") as psp:

            # ---- constants (loaded once, outside the repeat loop) ----
            w1_s = cpool.tile([P, P], BF16)
            nc.sync.dma_start(out=w1_s[:], in_=w1_d[:, :])
            w2_s = cpool.tile([P, P], BF16)
            nc.sync.dma_start(out=w2_s[:], in_=w2_d[:, :])
            wf2o_s = cpool.tile([P, P], BF16)
            nc.sync.dma_start(out=wf2o_s[:], in_=wf2o_d[:, :])
            wd_s = cpool.tile([P, P], BF16)
            nc.sync.dma_start(out=wd_s[:], in_=wd_d[:, :])
            b1_s = cpool.tile([P, 1], F32)
            nc.sync.dma_start(out=b1_s[:], in_=b1_d[:, :])
            bf2o_s = cpool.tile([P, 1], F32)
            nc.sync.dma_start(out=bf2o_s[:], in_=bf2o_d[:, :])
            bdp_s = cpool.tile([P, P], F32)
            nc.sync.dma_start(out=bdp_s[:], in_=bdp_d[:, :])
            iota_s = cpool.tile([P, P], BF16)
            nc.sync.dma_start(out=iota_s[:], in_=iota_d[:, :])
            segl_s = cpool.tile([P, NTILE], BF16)
            nc.sync.dma_start(out=segl_s[:], in_=segl_d[:, :])
            xb_s = cpool.tile([P, NW, P], BF16)
            nc.sync.dma_start(out=xb_s[:], in_=xb_d[:, :, :])
            half_s = cpool.tile([P, 1], F32)
            nc.vector.memset(half_s[:], 0.5)

            convT = cpool.tile([P, BAND], BF16)

            oh_eng = nc.gpsimd if onehot_engine == "gpsimd" else nc.vector

            def body():
                dks, xjs, hs, wsbs, wfs, ohs = {}, {}, {}, {}, {}, {}
                cvs = {}

                def dma_in(s):
                    dk = iop.tile([P, GW], BF16, tag="dk", bufs=3)
                    nc.sync.dma_start(out=dk[:], in_=dijk_d[s])
                    xjt = iop.tile([P, GW], BF16, tag="xj", bufs=5)
                    nc.sync.dma_start(out=xjt[:], in_=xj_d[s])
                    dks[s], xjs[s] = dk, xjt

                def onehot(s):
                    oh = midp.tile([P, G, P], BF16, tag="oh", bufs=5)
                    tl = s * G
                    oh_eng.tensor_tensor(
                        out=oh[:],
                        in0=segl_s[:, tl:tl + G].unsqueeze(2)
                            .to_broadcast([P, G, P]),
                        in1=iota_s[:].unsqueeze(1).to_broadcast([P, G, P]),
                        op=ALU.is_equal)
                    ohs[s] = oh

                pairs = {}

                def stage_mm_exp(p, s1, s2):
                    """mm1 for group s1 and mm2 for group s2 share one PSUM
                    tile; one Exp covers both (2048 wide). The matching Ln
                    runs one step later (stage_ln) so no ACT instruction
                    reads the write of its immediate predecessor."""
                    a12 = psp.tile([P, 2 * GW], F32, tag="a12", bufs=1)
                    lo, hi = (0 if s1 is not None else GW,
                              2 * GW if s2 is not None else GW)
                    if s1 is not None:
                        dk = dks[s1]
                        for hf in range(2):
                            c0 = hf * 512
                            nc.tensor.matmul(out=a12[:, c0:c0 + 512],
                                             lhsT=w1_s[:],
                                             rhs=dk[:, c0:c0 + 512],
                                             start=True, stop=True)
                    if s2 is not None:
                        h = hs[s2]
                        for c in range(G):
                            nc.tensor.matmul(
                                out=a12[:, GW + c * P:GW + (c + 1) * P],
                                lhsT=h[0][:, h[1] + c * P:h[1] + (c + 1) * P],
                                rhs=w2_s[:], start=True, stop=True)
                        del hs[s2]
                    e12 = midp.tile([P, 2 * GW], BF16, tag="e12", bufs=3)
                    nc.scalar.activation(out=e12[:, lo:hi], in_=a12[:, lo:hi],
                                         func=AF.Exp)
                    pairs[p] = (e12, lo, hi, s1, s2)

                def stage_ln(p):
                    e12, lo, hi, s1, s2 = pairs.pop(p)
                    hw12 = midp.tile([P, 2 * GW], BF16, tag="hw12", bufs=3)
                    nc.scalar.activation(out=hw12[:, lo:hi],
                                         in_=e12[:, lo:hi],
                                         func=AF.Ln, scale=0.5,
                                         bias=half_s[:, :1])
                    if s1 is not None:
                        hs[s1] = (hw12, 0)       # h(s1) = hw12[:, :GW]
                    if s2 is not None:
                        wsbs[s2] = (hw12, GW)    # w(s2) = hw12[:, GW:]

                def wfmul(s):
                    wf = midp.tile([P, GW], BF16, tag="wf", bufs=3)
                    w = wsbs[s]
                    nc.vector.tensor_tensor(
                        out=wf[:], in0=w[0][:, w[1]:w[1] + GW],
                        in1=xjs[s][:], op=ALU.mult)
                    wfs[s] = wf
                    del wsbs[s], xjs[s]

                state = {"flushed": 0, "next_macro": 0}

                def scatter(s):
                    wf, oh = wfs[s], ohs[s]
                    for c in range(G):
                        t = s * G + c
                        sl = int(slot_of[t])
                        if t == FLUSH[sl]:
                            cvs[sl] = psp.tile([P, P], F32, tag="cv",
                                               bufs=2, name=f"cv{sl % 2}")
                        nc.tensor.matmul(out=cvs[sl][:],
                                         lhsT=wf[:, c * P:(c + 1) * P],
                                         rhs=oh[:, c, :],
                                         start=(t == FLUSH[sl]),
                                         stop=(t == FLUSH[sl + 1] - 1))
                        if t == FLUSH[sl + 1] - 1:
                            nc.vector.tensor_copy(
                                out=convT[:, sl * P:(sl + 1) * P],
                                in_=cvs[sl][:])
                            del cvs[sl]
                            state["flushed"] = sl + 1
                    del wfs[s], ohs[s]

                def atom_macro(m0, mw):
                    aw = mw * P
                    u = psp.tile([P, GW], F32, tag="a12", bufs=1, name="u")
                    for c0 in range(0, aw, 512):
                        cw = min(512, aw - c0)
                        nc.tensor.matmul(
                            out=u[:, c0:c0 + cw], lhsT=wf2o_s[:],
                            rhs=convT[:, m0 * P + c0:m0 * P + c0 + cw],
                            start=True, stop=True)
                    ec = midp.tile([P, GW], BF16, tag="ec", bufs=2, name="ec")
                    nc.scalar.activation(out=ec[:, :aw], in_=u[:, :aw],
                                         func=AF.Exp, bias=bf2o_s[:, :1])
                    cT = midp.tile([P, GW], BF16, tag="cT", bufs=2, name="cT")
                    nc.scalar.activation(out=cT[:, :aw], in_=ec[:, :aw],
                                         func=AF.Ln, scale=0.5,
                                         bias=half_s[:, :1])
                    vps = psp.tile([P, G, P], F32, tag="a2", bufs=1,
                                   name="vps")
                    for c in range(mw):
                        nc.tensor.matmul(out=vps[:, c, :],
                                         lhsT=cT[:, c * P:(c + 1) * P],
                                         rhs=wd_s[:], start=True, stop=True)
                    vsb = midp.tile([P, G, P], BF16, tag="vsb", bufs=2,
                                    name="vsb")
                    nc.vector.tensor_tensor(
                        out=vsb[:, :mw, :], in0=vps[:, :mw, :],
                        in1=bdp_s[:].unsqueeze(1).to_broadcast([P, mw, P]),
                        op=ALU.add)
                    ysb = midp.tile([P, G, P], BF16, tag="ysb", bufs=2,
                                    name="ysb")
                    nc.vector.tensor_tensor(
                        out=ysb[:, :mw, :], in0=vps[:, :mw, :],
                        in1=xb_s[:, m0:m0 + mw, :],
                        op=ALU.add)
                    nc.sync.dma_start(
                        out=v_d[:, m0:m0 + mw, :],
                        in_=vsb[:, :mw, :])
                    nc.sync.dma_start(
                        out=y_d[:, m0:m0 + mw, :],
                        in_=ysb[:, :mw, :])

                def maybe_atom():
                    # emit atom-stage macro-chunks as soon as their window
                    # slots are flushed (hides the atom stage under the
                    # ACT-bound window pipeline).
                    while True:
                        m0 = state["next_macro"] * G
                        if m0 >= NW:
                            return
                        mw = min(G, NW - m0)
                        if state["flushed"] < m0 + mw:
                            return
                        atom_macro(m0, mw)
                        state["next_macro"] += 1

                for s in range(NG + 5):
                    if s < NG:
                        dma_in(s)
                    if 0 <= s - 1 < NG:
                        onehot(s - 1)
                    # pair p=s: mm1(s-1) + mm2(s-3) + Exp; its Ln runs next
                    # step, so consecutive ACT ops never have a RAW dep.
                    s1 = s - 1 if 0 <= s - 1 < NG else None
                    s2 = s - 3 if 0 <= s - 3 < NG else None
                    if s1 is not None or s2 is not None:
                        stage_mm_exp(s, s1, s2)
                    if s - 1 in pairs:
                        stage_ln(s - 1)
                    if 0 <= s - 4 < NG:
                        wfmul(s - 4)
                    if 0 <= s - 5 < NG:
                        scatter(s - 5)
                        maybe_atom()

            if repeat == 1:
                body()
            else:
                with tc.For_i(0, repeat, 1):
                    body()

    nc.compile()
    _cache[key] = nc
    return nc


def _preprocess(x, f, dijk, idx_j, seg_i):
    """Host-side sharding: deal the global 128-atom windows across cores by
    descending edge load (round-robin), so per-slot capacities (= cross-core
    max) hug the load distribution. Returns (in_maps, metas)."""
    E = dijk.shape[0]
    seg_i = np.asarray(seg_i, dtype=np.int64)
    idx_j = np.asarray(idx_j, dtype=np.int64)

    NWIN_G = (N_ATOMS + P - 1) // P
    bounds = np.searchsorted(seg_i, np.arange(0, (NWIN_G + 1) * P, P))
    wtiles = (bounds[1:] - bounds[:-1] + P - 1) // P  # tiles per window
    order_g = np.argsort(-wtiles, kind="stable")
    NW = (NWIN_G + NCORES - 1) // NCORES

    deal = -np.ones((NCORES, NW), dtype=np.int64)
    for idx, w in enumerate(order_g):
        deal[idx % NCORES, idx // NCORES] = w

    C = np.ones(NW, dtype=np.int64)
    for s in range(NW):
        for k in range(NCORES):
            w = deal[k, s]
            if w >= 0:
                C[s] = max(C[s], wtiles[w])
    NTILE = int(C.sum())
    C[-1] += (-NTILE) % G
    NTILE = int(C.sum())
    FLUSH = tuple(int(v) for v in np.concatenate([[0], np.cumsum(C)]))

    E_pad = NTILE * P
    NG = NTILE // G
    in_maps = []
    metas = []
    for k in range(NCORES):
        oc = np.zeros(E_pad, dtype=np.int64)
        valid = np.zeros(E_pad, dtype=bool)
        wbase = np.zeros(E_pad, dtype=np.int64)
        for sl in range(NW):
            w = int(deal[k, sl])
            if w < 0:
                continue
            es, ee = int(bounds[w]), int(bounds[w + 1])
            n = ee - es
            o = FLUSH[sl] * P
            oc[o:o + n] = np.arange(es, ee)
            valid[o:o + n] = True
            wbase[o:FLUSH[sl + 1] * P] = w * P

        dmat = dijk[oc]
        dmat[~valid] = 0.0
        dijk_blk = np.ascontiguousarray(
            dmat.reshape(NG, GW, P).transpose(0, 2, 1)
        ).astype(ml_dtypes.bfloat16)

        xmat = f[idx_j[oc]]
        xmat[~valid] = 0.0
        xj_blk = np.ascontiguousarray(
            xmat.reshape(NG, G, P, P).transpose(0, 2, 1, 3).reshape(NG, P, GW)
        ).astype(ml_dtypes.bfloat16)

        segl_flat = np.where(valid, seg_i[oc] - wbase, -1).astype(np.float32)
        segl = np.ascontiguousarray(
            segl_flat.reshape(NTILE, P).T).astype(ml_dtypes.bfloat16)

        in_maps.append({"dijk": dijk_blk, "xj": xj_blk, "segl": segl})
        metas.append(deal[k])
    return in_maps, metas, NW, NTILE, FLUSH


def prepare(x, dijk, idx_j, seg_i, seg_j, seg_i_sum,
            W_f1, b_f1, W_f2, b_f2,
            W_in2fac, W_fac2out, b_fac2out,
            W_dense, b_dense):
    x = np.asarray(x, dtype=np.float32)
    dijk = np.asarray(dijk, dtype=np.float32)

    assert not np.any(np.asarray(b_f2)), \
        "b_f2 != 0 not supported by this build"
    assert not np.any(np.asarray(b_f1)), \
        "b_f1 != 0 not supported by this build (paired Exp has no bias slot)"

    f = x @ np.asarray(W_in2fac, dtype=np.float32)
    in_maps, metas, NW, NTILE, FLUSH = _preprocess(x, f, dijk, idx_j, seg_i)

    W_dense = np.asarray(W_dense, dtype=np.float32)
    bdp = np.asarray(b_dense, dtype=np.float32)

    consts = {
        "w1": np.asarray(W_f1, np.float32).astype(ml_dtypes.bfloat16),
        "w2": np.asarray(W_f2, np.float32).astype(ml_dtypes.bfloat16),
        "wf2o": np.asarray(W_fac2out, np.float32).astype(ml_dtypes.bfloat16),
        "wd": W_dense.astype(ml_dtypes.bfloat16),
        "b1": np.asarray(b_f1, np.float32).reshape(P, 1),
        "bf2o": np.asarray(b_fac2out, np.float32).reshape(P, 1),
        "bdp": np.broadcast_to(bdp[None, :], (P, P)).copy(),
        "iota": np.broadcast_to(
            np.arange(P, dtype=np.float32)[None, :],
            (P, P)).astype(ml_dtypes.bfloat16).copy(),
    }
    for k, m in enumerate(in_maps):
        deal_k = metas[k]
        # xb[p, sl, j] = x[128*deal[sl] + p, j] + bdp[j]
        xb = np.zeros((P, NW, P), dtype=np.float32)
        for sl in range(NW):
            w = int(deal_k[sl])
            if w < 0:
                continue
            r0 = w * P
            n = min(P, N_ATOMS - r0)
            xb[:n, sl, :] = x[r0:r0 + n] + bdp[None, :]
        m["xb"] = xb.astype(ml_dtypes.bfloat16)
        m.update(consts)
    return (in_maps, metas, NW, NTILE, FLUSH)


def run_prepared(prepared, _repeat=1):
    in_maps, metas, NW, NTILE, FLUSH = prepared
    nc = _build_nc(NW, NTILE, FLUSH, repeat=_repeat)
    res = run_bass_kernel_spmd(nc, in_maps, core_ids=list(range(NCORES)))

    y = np.empty((N_ATOMS, P), dtype=np.float32)
    v = np.empty((N_ATOMS, P), dtype=np.float32)
    for k, deal_k in enumerate(metas):
        yk = np.asarray(res.results[k]["y"], dtype=np.float32)
        vk = np.asarray(res.results[k]["v"], dtype=np.float32)
        for sl in range(NW):
            w = int(deal_k[sl])
            if w < 0:
                continue
            r0 = w * P
            n = min(P, N_ATOMS - r0)
            y[r0:r0 + n] = yk[:n, sl, :]
            v[r0:r0 + n] = vk[:n, sl, :]
    return (y, v)


def kernel(**inputs):
    return run_prepared(prepare(**inputs))
